# revision 19
# baseline (speedup 1.0000x reference)
"""Trainium2 Bass kernel for nn_CrossAttention (sparse attention).

Shapes (hardcoded): N=8, QL=256, KL=64, S=64, D=128, H=8.
Sharding: data-parallel over N across the 8 NeuronCores (one batch
element per core; no collectives needed — fc_out is local per batch).

Math (per batch n):
  q = query @ Wq.T                      [QL, D]
  k = keys @ Wk.T, v = values @ Wv.T    [KL*S, D]
  energy[q, ls] = q @ k.T               (head-independent)
  E = exp(energy / sqrt(D))             (head-independent; no max-sub
                                         needed: energy/sqrt(D) ~ N(0,1))
  per head h:  W = mask_h * E           (masked exp weights)
               denom[s, q] = sum_l W    (softmax over KL axis only)
               A = W / denom            (attention)
               outT_h[d, q] = v.T @ A
  y = concat_h(out_h) @ Wo.T + bo       [QL, D]

On-chip layout: the attention contraction runs over ls = l*64+s (4096)
in 32 chunks of 128 partitions.  All transposes ride the DMA xbar
(2-byte path) after casting to bf16 during the DMA load itself.  The
softmax denominator is computed on the TensorEngine with a constant
selector matrix sel2[p, m] = (p%64 == m%64), which reduces over l and
replicates the [s, q] result to 128 rows in one shot.
"""

import numpy as np

N, QL, KL, S, D, H = 8, 256, 64, 64, 128, 8
LS = KL * S          # 4096
NCHUNK = LS // 128   # 32
N_CORES = 8

_cache = {}
last_exec_time_ns = None


def _build():
    from contextlib import ExitStack
    import concourse.bacc as bacc
    import concourse.mybir as mybir
    import concourse.tile as tile
    from concourse import masks

    f32 = mybir.dt.float32
    bf16 = mybir.dt.bfloat16
    i32 = mybir.dt.int32

    nc = bacc.Bacc("TRN2", target_bir_lowering=False, debug=False,
                   enable_asserts=False, num_devices=N_CORES)

    values = nc.dram_tensor("values", [LS, D], f32, kind="ExternalInput").ap()
    keys = nc.dram_tensor("keys", [LS, D], f32, kind="ExternalInput").ap()
    query = nc.dram_tensor("query", [QL, D], f32, kind="ExternalInput").ap()
    mask = nc.dram_tensor("mask", [H, QL, LS], i32, kind="ExternalInput").ap()
    Wq = nc.dram_tensor("Wq", [D, D], f32, kind="ExternalInput").ap()
    Wk = nc.dram_tensor("Wk", [D, D], f32, kind="ExternalInput").ap()
    Wv = nc.dram_tensor("Wv", [D, D], f32, kind="ExternalInput").ap()
    Wo = nc.dram_tensor("Wo", [D, H * D], f32, kind="ExternalInput").ap()
    bo = nc.dram_tensor("bo", [D], f32, kind="ExternalInput").ap()
    y = nc.dram_tensor("y", [QL, D], f32, kind="ExternalOutput").ap()

    inv_scale = float(1.0 / np.sqrt(D))

    with tile.TileContext(nc) as tc, ExitStack() as ctx:
        hwdge = (nc.sync, nc.scalar)
        const_p = ctx.enter_context(tc.tile_pool(name="const", bufs=1))
        persist = ctx.enter_context(tc.tile_pool(name="persist", bufs=1))
        pst = ctx.enter_context(tc.tile_pool(name="pst", bufs=4, space="PSUM"))
        psm = ctx.enter_context(tc.tile_pool(name="psm", bufs=4, space="PSUM"))

        # ---- constants ----
        ident = const_p.tile([128, 128], f32, tag="identf")
        masks.make_identity(nc, ident[:])
        sel2f = const_p.tile([128, 128], f32, tag="sel2f")
        nc.gpsimd.memset(sel2f[:], 0.0)
        for base in (-64, 0, 64):
            nc.gpsimd.affine_select(
                out=sel2f[:], in_=sel2f[:],
                compare_op=mybir.AluOpType.not_equal, fill=1.0,
                base=base, pattern=[[-1, 128]], channel_multiplier=1)
        ident_b = const_p.tile([128, 128], bf16, tag="identb")
        nc.vector.tensor_copy(ident_b[:], ident[:])
        sel2 = const_p.tile([128, 128], bf16, tag="sel2")
        nc.vector.tensor_copy(sel2[:], sel2f[:])
        ones1 = const_p.tile([1, 128], f32, tag="ones1")
        nc.vector.memset(ones1[:], 1.0)
        bo_sb = const_p.tile([1, 128], f32, tag="bo")
        nc.sync.dma_start(bo_sb[:], bo[None, :])

        # ---- persistent intermediates ----
        q_projT = persist.tile([128, QL], bf16, tag="qprojT")      # [e, q]
        k_projT = persist.tile([128, LS], bf16, tag="kprojT")      # [e, c*128+ls%]
        v_proj = persist.tile([128, LS], bf16, tag="vproj")        # [ls%, c*128+e]
        E = persist.tile([128, NCHUNK * QL], bf16, tag="E")        # [ls%, c*256+q]
        WoT = persist.tile([128, H * D], f32, tag="WoT")           # [d_h, h*128+e]
        outT = persist.tile([128, H * QL], f32, tag="outT")        # [d, h*256+q]

        E3 = E[:].rearrange("p (c q) -> p c q", q=QL)

        maskf_p = ctx.enter_context(tc.tile_pool(name="maskf", bufs=4))
        wt_p = ctx.enter_context(tc.tile_pool(name="wt", bufs=2))
        r_p = ctx.enter_context(tc.tile_pool(name="r", bufs=2))

        with tc.tile_pool(name="prolog", bufs=1) as prolog:
            # -- small weights: PE transposes (f32) --
            wqt = prolog.tile([128, 128], f32, tag="wqt")
            wkt = prolog.tile([128, 128], bf16, tag="wkt")
            wvt = prolog.tile([128, 128], bf16, tag="wvt")
            for wdram, wt in ((Wq, wqt), (Wk, wkt), (Wv, wvt)):
                wnat = prolog.tile([128, 128], f32, tag="wnat")
                nc.sync.dma_start(wnat[:], wdram[:])
                ps = pst.tile([128, 128], f32, tag="t")
                nc.tensor.transpose(ps[:], wnat[:], ident[:])
                nc.scalar.copy(wt[:], ps[:])

            # query -> queryT [d, q] -> q_projT (bf16)
            qnat = prolog.tile([128, QL], f32, tag="qnat")
            nc.sync.dma_start(
                qnat[:].rearrange("p (u d) -> p u d", d=128),
                query[:].rearrange("(u p) d -> p u d", p=128))
            queryT = prolog.tile([128, QL], f32, tag="queryT")
            for u in range(2):
                ps = pst.tile([128, 128], f32, tag="t")
                nc.tensor.transpose(ps[:], qnat[:, u * 128:(u + 1) * 128], ident[:])
                nc.scalar.copy(queryT[:, u * 128:(u + 1) * 128], ps[:])
            psq = psm.tile([128, QL], f32, tag="m")
            nc.tensor.matmul(psq[:], wqt[:], queryT[:])
            nc.scalar.copy(q_projT[:], psq[:])

            # -- keys: f32 load -> DVE cast -> xbar transpose -> k_projT --
            knat32 = prolog.tile([128, LS], f32, tag="stage32")
            nc.sync.dma_start(
                knat32[:].rearrange("p (c d) -> p c d", d=128),
                keys[:].rearrange("(c p) d -> p c d", p=128))
            knat = prolog.tile([128, LS], bf16, tag="knat")
            nc.vector.tensor_copy(knat[:], knat32[:])
            keysT = prolog.tile([128, LS], bf16, tag="keysT")
            nc.sync.dma_start_transpose(
                keysT[:].rearrange("p (c q) -> p c q", q=128), knat[:])
            for j in range(LS // 512):
                ps = psm.tile([128, 512], f32, tag="m")
                nc.tensor.matmul(ps[:], wkt[:], keysT[:, j * 512:(j + 1) * 512])
                nc.vector.tensor_copy(k_projT[:, j * 512:(j + 1) * 512], ps[:])

            # energy two chunks at a time -> one exp evac per psum bank
            for c2 in range(NCHUNK // 2):
                ps = psm.tile([128, 512], f32, tag="m")
                for k in range(2):
                    c = 2 * c2 + k
                    nc.tensor.matmul(ps[:, k * QL:(k + 1) * QL],
                                     k_projT[:, c * 128:(c + 1) * 128],
                                     q_projT[:])
                nc.scalar.activation(E[:, c2 * 512:(c2 + 1) * 512], ps[:],
                                     mybir.ActivationFunctionType.Exp,
                                     scale=inv_scale)

            # -- values: f32 load -> DVE cast -> xbar transpose -> v_proj --
            vnat32 = prolog.tile([128, LS], f32, tag="stage32")
            nc.sync.dma_start(
                vnat32[:].rearrange("p (c d) -> p c d", d=128),
                values[:].rearrange("(c p) d -> p c d", p=128))
            vnat = prolog.tile([128, LS], bf16, tag="vnat")
            nc.vector.tensor_copy(vnat[:], vnat32[:])
            valuesT = prolog.tile([128, LS], bf16, tag="valuesT")
            nc.sync.dma_start_transpose(
                valuesT[:].rearrange("p (c q) -> p c q", q=128), vnat[:])
            for c in range(NCHUNK):
                ps = pst.tile([128, 128], f32, tag="t")
                nc.tensor.matmul(ps[:], valuesT[:, c * 128:(c + 1) * 128], wvt[:])
                nc.vector.tensor_copy(v_proj[:, c * 128:(c + 1) * 128], ps[:])

            # -- Wo -> WoT (f32, PE transposes) --
            wonat = prolog.tile([128, H * D], f32, tag="wonat")
            nc.sync.dma_start(wonat[:], Wo[:])
            for h in range(H):
                ps = pst.tile([128, 128], f32, tag="t")
                nc.tensor.transpose(ps[:], wonat[:, h * 128:(h + 1) * 128], ident[:])
                nc.scalar.copy(WoT[:, h * 128:(h + 1) * 128], ps[:])

        # ---- per-head stream ----
        for h in range(H):
            # WT goes through three lives in place:
            #   transposed mask -> W = mask*E -> A = W/denom
            # layout [ls%, c*256 + u*128 + q] (matches E)
            WT = wt_p.tile([128, NCHUNK * QL], bf16, tag="wt", name="wt")
            for u in range(2):
                # DMA with int32 -> bf16 cast (SWDGE)
                mf = maskf_p.tile([128, LS], bf16, tag="maskf")
                nc.gpsimd.dma_start(mf[:], mask[h, u * 128:(u + 1) * 128, :])
                # PE transpose [q_half, ls-chunk] -> [ls%, q_half] per chunk
                for c in range(NCHUNK):
                    tps = pst.tile([128, 128], bf16, tag="t", name="tps")
                    nc.tensor.transpose(tps[:], mf[:, c * 128:(c + 1) * 128],
                                        ident_b[:])
                    nc.scalar.copy(
                        WT[:, c * QL + u * 128: c * QL + (u + 1) * 128], tps[:])

            nc.vector.tensor_mul(WT[:], WT[:], E[:])
            ps_den = psm.tile([128, QL], f32, tag="m")
            for c in range(NCHUNK):
                nc.tensor.matmul(ps_den[:], sel2[:], WT[:, c * QL:(c + 1) * QL],
                                 start=(c == 0), stop=(c == NCHUNK - 1))
            R = r_p.tile([128, QL], f32, tag="r")
            nc.vector.reciprocal_approx_fast(R[:], ps_den[:])
            Rb = r_p.tile([128, QL], bf16, tag="rb")
            nc.vector.tensor_copy(Rb[:], R[:])

            WT3 = WT[:].rearrange("p (c q) -> p c q", q=QL)
            nc.vector.tensor_mul(
                WT3, WT3, Rb[:, None, :].to_broadcast((128, NCHUNK, QL)))
            ps_o = psm.tile([128, QL], f32, tag="m")
            for c in range(NCHUNK):
                nc.tensor.matmul(ps_o[:], v_proj[:, c * 128:(c + 1) * 128],
                                 WT[:, c * QL:(c + 1) * QL],
                                 start=(c == 0), stop=(c == NCHUNK - 1))
            nc.scalar.copy(outT[:, h * QL:(h + 1) * QL], ps_o[:])

        # ---- fc_out: y[q, e] = outT.T @ WoT + bo ----
        with tc.tile_pool(name="fin", bufs=2) as fin:
            for u in range(2):
                ps = psm.tile([128, 128], f32, tag="m")
                for h in range(H):
                    nc.tensor.matmul(
                        ps[:], outT[:, h * QL + u * 128: h * QL + (u + 1) * 128],
                        WoT[:, h * 128:(h + 1) * 128],
                        start=(h == 0), stop=False)
                nc.tensor.matmul(ps[:], ones1[:1, :], bo_sb[:1, :],
                                 start=False, stop=True)
                ysb = fin.tile([128, 128], f32, tag="y")
                nc.vector.tensor_copy(ysb[:], ps[:])
                nc.sync.dma_start(y[u * 128:(u + 1) * 128, :], ysb[:])

    nc.compile()
    return nc


def kernel(values, keys, query, mask, Wq, Wk, Wv, Wo, bo):
    global last_exec_time_ns
    from concourse import bass_utils

    if "nc" not in _cache:
        _cache["nc"] = _build()
    nc = _cache["nc"]

    values = np.ascontiguousarray(np.asarray(values, dtype=np.float32))
    keys = np.ascontiguousarray(np.asarray(keys, dtype=np.float32))
    query = np.ascontiguousarray(np.asarray(query, dtype=np.float32))
    mask = np.ascontiguousarray(np.asarray(mask, dtype=np.int32))
    Wq = np.ascontiguousarray(np.asarray(Wq, dtype=np.float32))
    Wk = np.ascontiguousarray(np.asarray(Wk, dtype=np.float32))
    Wv = np.ascontiguousarray(np.asarray(Wv, dtype=np.float32))
    Wo = np.ascontiguousarray(np.asarray(Wo, dtype=np.float32))
    bo = np.ascontiguousarray(np.asarray(bo, dtype=np.float32))

    in_maps = []
    for n in range(N_CORES):
        in_maps.append({
            "values": values[n].reshape(LS, D),
            "keys": keys[n].reshape(LS, D),
            "query": query[n],
            "mask": mask[n].reshape(H, QL, LS),
            "Wq": Wq, "Wk": Wk, "Wv": Wv, "Wo": Wo, "bo": bo,
        })

    res = bass_utils.run_bass_kernel_spmd(nc, in_maps,
                                          core_ids=list(range(N_CORES)))
    last_exec_time_ns = res.exec_time_ns
    out = np.stack([res.results[n]["y"] for n in range(N_CORES)], axis=0)
    return out.astype(np.float32)


# revision 20
# speedup vs baseline: 1.4857x; 1.4857x over previous
"""Trainium2 Bass kernel for nn_CrossAttention (sparse attention).

Shapes (hardcoded): N=8, QL=256, KL=64, S=64, D=128, H=8.
Sharding: data-parallel over N across the 8 NeuronCores (one batch
element per core; no collectives needed — fc_out is local per batch).

Math (per batch n):
  q = query @ Wq.T                      [QL, D]
  k = keys @ Wk.T, v = values @ Wv.T    [KL*S, D]
  energy[q, ls] = q @ k.T               (head-independent)
  E = exp(energy / sqrt(D))             (head-independent; no max-sub
                                         needed: energy/sqrt(D) ~ N(0,1))
  per head h:  W = mask_h * E           (masked exp weights)
               denom[s, q] = sum_l W    (softmax over KL axis only)
               A = W / denom            (attention)
               outT_h[d, q] = v.T @ A
  y = concat_h(out_h) @ Wo.T + bo       [QL, D]

On-chip layout: the attention contraction runs over ls = l*64+s (4096)
in 32 chunks of 128 partitions.  All transposes ride the DMA xbar
(2-byte path) after casting to bf16 during the DMA load itself.  The
softmax denominator is computed on the TensorEngine with a constant
selector matrix sel2[p, m] = (p%64 == m%64), which reduces over l and
replicates the [s, q] result to 128 rows in one shot.
"""

import numpy as np

N, QL, KL, S, D, H = 8, 256, 64, 64, 128, 8
LS = KL * S          # 4096
NCHUNK = LS // 128   # 32
N_CORES = 8

_cache = {}
last_exec_time_ns = None


def _build():
    from contextlib import ExitStack
    import concourse.bacc as bacc
    import concourse.mybir as mybir
    import concourse.tile as tile
    from concourse import masks

    f32 = mybir.dt.float32
    bf16 = mybir.dt.bfloat16
    i32 = mybir.dt.int32

    nc = bacc.Bacc("TRN2", target_bir_lowering=False, debug=False,
                   enable_asserts=False, num_devices=N_CORES)

    values = nc.dram_tensor("values", [LS, D], f32, kind="ExternalInput").ap()
    keys = nc.dram_tensor("keys", [LS, D], f32, kind="ExternalInput").ap()
    query = nc.dram_tensor("query", [QL, D], f32, kind="ExternalInput").ap()
    mask = nc.dram_tensor("mask", [H, QL, LS], i32, kind="ExternalInput").ap()
    Wq = nc.dram_tensor("Wq", [D, D], f32, kind="ExternalInput").ap()
    Wk = nc.dram_tensor("Wk", [D, D], f32, kind="ExternalInput").ap()
    Wv = nc.dram_tensor("Wv", [D, D], f32, kind="ExternalInput").ap()
    Wo = nc.dram_tensor("Wo", [D, H * D], f32, kind="ExternalInput").ap()
    bo = nc.dram_tensor("bo", [D], f32, kind="ExternalInput").ap()
    y = nc.dram_tensor("y", [QL, D], f32, kind="ExternalOutput").ap()

    inv_scale = float(1.0 / np.sqrt(D))

    with tile.TileContext(nc) as tc, ExitStack() as ctx:
        hwdge = (nc.sync, nc.scalar)
        const_p = ctx.enter_context(tc.tile_pool(name="const", bufs=1))
        persist = ctx.enter_context(tc.tile_pool(name="persist", bufs=1))
        pst = ctx.enter_context(tc.tile_pool(name="pst", bufs=4, space="PSUM"))
        psm = ctx.enter_context(tc.tile_pool(name="psm", bufs=4, space="PSUM"))

        # ---- constants ----
        ident = const_p.tile([128, 128], f32, tag="identf")
        masks.make_identity(nc, ident[:])
        sel2f = const_p.tile([128, 128], f32, tag="sel2f")
        nc.gpsimd.memset(sel2f[:], 0.0)
        for base in (-64, 0, 64):
            nc.gpsimd.affine_select(
                out=sel2f[:], in_=sel2f[:],
                compare_op=mybir.AluOpType.not_equal, fill=1.0,
                base=base, pattern=[[-1, 128]], channel_multiplier=1)
        ident_b = const_p.tile([128, 128], bf16, tag="identb")
        nc.vector.tensor_copy(ident_b[:], ident[:])
        sel2 = const_p.tile([128, 128], bf16, tag="sel2")
        nc.vector.tensor_copy(sel2[:], sel2f[:])
        ones1 = const_p.tile([1, 128], f32, tag="ones1")
        nc.vector.memset(ones1[:], 1.0)
        bo_sb = const_p.tile([1, 128], f32, tag="bo")
        nc.sync.dma_start(bo_sb[:], bo[None, :])

        # ---- persistent intermediates ----
        q_projT = persist.tile([128, QL], bf16, tag="qprojT")      # [e, q]
        k_projT = persist.tile([128, LS], bf16, tag="kprojT")      # [e, c*128+ls%]
        v_proj = persist.tile([128, LS], bf16, tag="vproj")        # [ls%, c*128+e]
        E = persist.tile([128, NCHUNK * QL], bf16, tag="E")        # [ls%, c*256+q]
        WoT = persist.tile([128, H * D], f32, tag="WoT")           # [d_h, h*128+e]
        outT = persist.tile([128, H * QL], f32, tag="outT")        # [d, h*256+q]

        E3 = E[:].rearrange("p (c q) -> p c q", q=QL)

        maskf_p = ctx.enter_context(tc.tile_pool(name="maskf", bufs=4))
        wt_p = ctx.enter_context(tc.tile_pool(name="wt", bufs=2))
        r_p = ctx.enter_context(tc.tile_pool(name="r", bufs=2))

        with tc.tile_pool(name="prolog", bufs=1) as prolog:
            # -- small weights: PE transposes (f32) --
            wqt = prolog.tile([128, 128], f32, tag="wqt")
            wkt = prolog.tile([128, 128], bf16, tag="wkt")
            wvt = prolog.tile([128, 128], bf16, tag="wvt")
            for wdram, wt in ((Wq, wqt), (Wk, wkt), (Wv, wvt)):
                wnat = prolog.tile([128, 128], f32, tag="wnat")
                nc.sync.dma_start(wnat[:], wdram[:])
                ps = pst.tile([128, 128], f32, tag="t")
                nc.tensor.transpose(ps[:], wnat[:], ident[:])
                nc.scalar.copy(wt[:], ps[:])

            # query -> queryT [d, q] -> q_projT (bf16)
            qnat = prolog.tile([128, QL], f32, tag="qnat")
            nc.sync.dma_start(
                qnat[:].rearrange("p (u d) -> p u d", d=128),
                query[:].rearrange("(u p) d -> p u d", p=128))
            queryT = prolog.tile([128, QL], f32, tag="queryT")
            for u in range(2):
                ps = pst.tile([128, 128], f32, tag="t")
                nc.tensor.transpose(ps[:], qnat[:, u * 128:(u + 1) * 128], ident[:])
                nc.scalar.copy(queryT[:, u * 128:(u + 1) * 128], ps[:])
            psq = psm.tile([128, QL], f32, tag="m")
            nc.tensor.matmul(psq[:], wqt[:], queryT[:])
            nc.scalar.copy(q_projT[:], psq[:])

            # -- keys: f32 load -> DVE cast -> xbar transpose -> k_projT --
            knat32 = prolog.tile([128, LS], f32, tag="stage32")
            nc.sync.dma_start(
                knat32[:].rearrange("p (c d) -> p c d", d=128),
                keys[:].rearrange("(c p) d -> p c d", p=128))
            knat = prolog.tile([128, LS], bf16, tag="knat")
            nc.vector.tensor_copy(knat[:], knat32[:])
            keysT = prolog.tile([128, LS], bf16, tag="keysT")
            nc.sync.dma_start_transpose(
                keysT[:].rearrange("p (c q) -> p c q", q=128), knat[:])
            for j in range(LS // 512):
                ps = psm.tile([128, 512], f32, tag="m")
                nc.tensor.matmul(ps[:], wkt[:], keysT[:, j * 512:(j + 1) * 512])
                nc.vector.tensor_copy(k_projT[:, j * 512:(j + 1) * 512], ps[:])

            # energy two chunks at a time -> one exp evac per psum bank
            for c2 in range(NCHUNK // 2):
                ps = psm.tile([128, 512], f32, tag="m")
                for k in range(2):
                    c = 2 * c2 + k
                    nc.tensor.matmul(ps[:, k * QL:(k + 1) * QL],
                                     k_projT[:, c * 128:(c + 1) * 128],
                                     q_projT[:])
                nc.scalar.activation(E[:, c2 * 512:(c2 + 1) * 512], ps[:],
                                     mybir.ActivationFunctionType.Exp,
                                     scale=inv_scale)

            # -- values: f32 load -> DVE cast -> xbar transpose -> v_proj --
            vnat32 = prolog.tile([128, LS], f32, tag="stage32")
            nc.sync.dma_start(
                vnat32[:].rearrange("p (c d) -> p c d", d=128),
                values[:].rearrange("(c p) d -> p c d", p=128))
            vnat = prolog.tile([128, LS], bf16, tag="vnat")
            nc.vector.tensor_copy(vnat[:], vnat32[:])
            valuesT = prolog.tile([128, LS], bf16, tag="valuesT")
            nc.sync.dma_start_transpose(
                valuesT[:].rearrange("p (c q) -> p c q", q=128), vnat[:])
            for c in range(NCHUNK):
                ps = pst.tile([128, 128], f32, tag="t")
                nc.tensor.matmul(ps[:], valuesT[:, c * 128:(c + 1) * 128], wvt[:])
                nc.vector.tensor_copy(v_proj[:, c * 128:(c + 1) * 128], ps[:])

            # -- Wo -> WoT (f32, PE transposes) --
            wonat = prolog.tile([128, H * D], f32, tag="wonat")
            nc.sync.dma_start(wonat[:], Wo[:])
            for h in range(H):
                ps = pst.tile([128, 128], f32, tag="t")
                nc.tensor.transpose(ps[:], wonat[:, h * 128:(h + 1) * 128], ident[:])
                nc.scalar.copy(WoT[:, h * 128:(h + 1) * 128], ps[:])

        # ---- per-head stream ----
        for h in range(H):
            # WT goes through three lives in place:
            #   transposed mask -> W = mask*E -> A = W/denom
            # layout [ls%, c*256 + u*128 + q] (matches E)
            WT = wt_p.tile([128, NCHUNK * QL], bf16, tag="wt", name="wt")
            for u in range(2):
                # DMA with int32 -> bf16 cast (SWDGE)
                mf = maskf_p.tile([128, LS], bf16, tag="maskf")
                nc.gpsimd.dma_start(mf[:], mask[h, u * 128:(u + 1) * 128, :])
                # PE transpose [q_half, ls-chunk] -> [ls%, q_half] per chunk;
                # 8 transposes share one PSUM bank, evacuated by one wide copy
                for c8 in range(NCHUNK // 8):
                    tps = pst.tile([128, 1024], bf16, tag="t", name="tps")
                    for j in range(8):
                        c = c8 * 8 + j
                        nc.tensor.transpose(tps[:, j * 128:(j + 1) * 128],
                                            mf[:, c * 128:(c + 1) * 128],
                                            ident_b[:])
                    dst = WT[:].rearrange("p (c q) -> p c q", q=QL)[
                        :, c8 * 8:(c8 + 1) * 8, u * 128:(u + 1) * 128]
                    nc.scalar.copy(dst, tps[:].rearrange(
                        "p (j q) -> p j q", q=128))

            nc.vector.tensor_mul(WT[:], WT[:], E[:])
            ps_den = psm.tile([128, QL], f32, tag="m")
            for c in range(NCHUNK):
                nc.tensor.matmul(ps_den[:], sel2[:], WT[:, c * QL:(c + 1) * QL],
                                 start=(c == 0), stop=(c == NCHUNK - 1))
            R = r_p.tile([128, QL], f32, tag="r")
            nc.vector.reciprocal_approx_fast(R[:], ps_den[:])
            Rb = r_p.tile([128, QL], bf16, tag="rb")
            nc.vector.tensor_copy(Rb[:], R[:])

            WT3 = WT[:].rearrange("p (c q) -> p c q", q=QL)
            nc.vector.tensor_mul(
                WT3, WT3, Rb[:, None, :].to_broadcast((128, NCHUNK, QL)))
            ps_o = psm.tile([128, QL], f32, tag="m")
            for c in range(NCHUNK):
                nc.tensor.matmul(ps_o[:], v_proj[:, c * 128:(c + 1) * 128],
                                 WT[:, c * QL:(c + 1) * QL],
                                 start=(c == 0), stop=(c == NCHUNK - 1))
            nc.scalar.copy(outT[:, h * QL:(h + 1) * QL], ps_o[:])

        # ---- fc_out: y[q, e] = outT.T @ WoT + bo ----
        with tc.tile_pool(name="fin", bufs=2) as fin:
            for u in range(2):
                ps = psm.tile([128, 128], f32, tag="m")
                for h in range(H):
                    nc.tensor.matmul(
                        ps[:], outT[:, h * QL + u * 128: h * QL + (u + 1) * 128],
                        WoT[:, h * 128:(h + 1) * 128],
                        start=(h == 0), stop=False)
                nc.tensor.matmul(ps[:], ones1[:1, :], bo_sb[:1, :],
                                 start=False, stop=True)
                ysb = fin.tile([128, 128], f32, tag="y")
                nc.vector.tensor_copy(ysb[:], ps[:])
                nc.sync.dma_start(y[u * 128:(u + 1) * 128, :], ysb[:])

    nc.compile()
    return nc


def kernel(values, keys, query, mask, Wq, Wk, Wv, Wo, bo):
    global last_exec_time_ns
    from concourse import bass_utils

    if "nc" not in _cache:
        _cache["nc"] = _build()
    nc = _cache["nc"]

    values = np.ascontiguousarray(np.asarray(values, dtype=np.float32))
    keys = np.ascontiguousarray(np.asarray(keys, dtype=np.float32))
    query = np.ascontiguousarray(np.asarray(query, dtype=np.float32))
    mask = np.ascontiguousarray(np.asarray(mask, dtype=np.int32))
    Wq = np.ascontiguousarray(np.asarray(Wq, dtype=np.float32))
    Wk = np.ascontiguousarray(np.asarray(Wk, dtype=np.float32))
    Wv = np.ascontiguousarray(np.asarray(Wv, dtype=np.float32))
    Wo = np.ascontiguousarray(np.asarray(Wo, dtype=np.float32))
    bo = np.ascontiguousarray(np.asarray(bo, dtype=np.float32))

    in_maps = []
    for n in range(N_CORES):
        in_maps.append({
            "values": values[n].reshape(LS, D),
            "keys": keys[n].reshape(LS, D),
            "query": query[n],
            "mask": mask[n].reshape(H, QL, LS),
            "Wq": Wq, "Wk": Wk, "Wv": Wv, "Wo": Wo, "bo": bo,
        })

    res = bass_utils.run_bass_kernel_spmd(nc, in_maps,
                                          core_ids=list(range(N_CORES)))
    last_exec_time_ns = res.exec_time_ns
    out = np.stack([res.results[n]["y"] for n in range(N_CORES)], axis=0)
    return out.astype(np.float32)
